# revision 24
# baseline (speedup 1.0000x reference)
"""Trainium2 Bass kernel for nn_BallQLossSeq (ball-query + grouped flow-norm loss).

Per core (1024 of 8192 query rows):
  1. PE: d2[i,j] via augmented matmul (16 contraction rows: hi/lo bf16 split of
     -2x, coords, |q|^2, |s|^2), 512-wide PSUM chunks.
  2. ACT: steep sigmoid (kappa=2^18) of (1-d2) -> ~0/1 hit indicator.
  3. DVE: tensor_tensor_scan (add, clamp via min 17) -> running hit count
     S in [0,17] as int16 keys.
  4. GPSIMD local_scatter: data = iota (t+1), keys = S -> slot[v] = position of
     hit v+1 (last-wins). Slots 0..15 = first-16 in-radius neighbor positions;
     rows with c<16 hits padded with slot[0] (first hit).
  5. dma_gather of 256B rows from a DRAM flow table, elementwise norm + sqrt,
     accumulate -> per-core scalar partial. Host sums partials / (S*N*K).
"""

import numpy as np

N = 8192
NCORES = 8
SLAB = N // NCORES          # 1024 query rows per core
NT = SLAB // 128            # 8 i-tiles per core
SEQ = 4
KNN = 16
NCHUNK = 16                 # j chunks of 512
CW = 512
KAPPA = 4194304.0
KROWS = 16                  # matmul contraction rows

_CACHE = {}


def _build_program():
    import os
    STAGE = int(os.environ.get("KSTAGE", "5"))
    import concourse.bass as bass
    import concourse.bacc as bacc
    import concourse.mybir as mybir
    import concourse.tile as tile
    import concourse.bass_isa as bass_isa

    f32 = mybir.dt.float32
    bf16 = mybir.dt.bfloat16
    i16 = mybir.dt.int16
    i32 = mybir.dt.int32
    Alu = mybir.AluOpType
    Act = mybir.ActivationFunctionType

    nc = bacc.Bacc()

    aug_rhs = nc.dram_tensor("aug_rhs", [KROWS, N], bf16, kind="ExternalInput")
    aug_lhsT = nc.dram_tensor("aug_lhsT", [KROWS, SLAB], bf16, kind="ExternalInput")
    flow_all = nc.dram_tensor("flow_all", [SEQ, N, 3], f32, kind="ExternalInput")
    flow_slab = nc.dram_tensor("flow_slab", [SEQ, SLAB, 3], f32, kind="ExternalInput")
    partial = nc.dram_tensor("partial", [1, 1], f32, kind="ExternalOutput")

    with tile.TileContext(nc) as tc:
        with (
            tc.tile_pool(name="const", bufs=1) as constp,
            tc.tile_pool(name="prep", bufs=1) as prep,
            tc.tile_pool(name="hpool", bufs=3) as hpool,
            tc.tile_pool(name="kpool", bufs=3) as kpool,
            tc.tile_pool(name="small", bufs=2) as small,
            tc.tile_pool(name="gath", bufs=2) as gath,
            tc.tile_pool(name="dram", bufs=1, space="DRAM") as drampool,
            tc.tile_pool(name="psum", bufs=6, space="PSUM") as psum,
            tc.tile_pool(name="tpsum", bufs=2, space="PSUM") as tpsum,
        ):
            # ---------------- constants ----------------
            iota1 = constp.tile([128, N], i16)           # values t+1
            nc.gpsimd.iota(iota1, pattern=[[1, N]], base=1, channel_multiplier=0)
            c17 = constp.tile([128, N], bf16)
            nc.gpsimd.memset(c17, 1792.0)
            iota16 = constp.tile([128, KNN], i32)
            nc.gpsimd.iota(iota16, pattern=[[1, KNN]], base=0, channel_multiplier=0)
            iota16f = constp.tile([128, KNN], f32)
            nc.vector.tensor_copy(iota16f, iota16)
            kbias = constp.tile([128, 1], f32)
            nc.gpsimd.memset(kbias, KAPPA)

            # ---------------- DRAM flow table [N, 12] (cols s*3+c) ------------
            table = drampool.tile([N, SEQ * 3], f32)
            for s in range(SEQ):
                nc.sync.dma_start(table[:, s * 3:(s + 1) * 3], flow_all[s])

            # ------------- aug matmul operands (host-prepped hi/lo bf16) ------
            rhs_t = constp.tile([KROWS, N], bf16)
            nc.sync.dma_start(rhs_t, aug_rhs[:])
            lhsT = constp.tile([KROWS, SLAB], bf16)
            nc.sync.dma_start(lhsT, aug_lhsT[:])

            # ------------- own flow vectors [128, NT, 12] (p = i%128) ----------
            own = constp.tile([128, NT, SEQ * 3], f32)
            for s in range(SEQ):
                nc.sync.dma_start(
                    own[:, :, 3 * s:3 * (s + 1)],
                    flow_slab[s].rearrange("(t p) c -> p t c", p=128))

            offs = constp.tile([128, NT * KNN], i32)
            tacc2 = constp.tile([128, SEQ], f32)

            # ================= main loop over i-tiles ==========================
            NSLOT = 1800
            for t in range(NT):
                h = hpool.tile([128, N], bf16, tag="h")
                for n in range(NCHUNK):
                    pd2 = psum.tile([128, CW], f32, tag="d2")
                    nc.tensor.matmul(pd2, lhsT[:, t * 128:(t + 1) * 128],
                                     rhs_t[:, n * CW:(n + 1) * CW],
                                     start=True, stop=True)
                    # h = sigmoid(kappa*(1 - d2))
                    nc.scalar.activation(h[:, n * CW:(n + 1) * CW], pd2,
                                         Act.Sigmoid, bias=kbias[:, :],
                                         scale=-KAPPA)
                # S[t] = min(1 + cumsum(h), 1792), with S[-1]=1 prepended
                sx = kpool.tile([128, N + 8], i32, tag="sx", bufs=1)
                nc.vector.memset(sx[:, 0:1], 1)
                nc.vector.tensor_tensor_scan(sx[:, 1:N + 1], h, c17,
                                             initial=1.3,
                                             op0=Alu.add, op1=Alu.min)
                # key = 1800*hit - S  (hit: unique slot 1800-S; miss: negative)
                keys = kpool.tile([128, N], i16, tag="keys")
                if STAGE < 2:
                    continue
                nc.vector.scalar_tensor_tensor(keys, sx[:, 0:N], 0.5,
                                               sx[:, 1:N + 1],
                                               op0=Alu.add, op1=Alu.is_lt)
                nc.vector.scalar_tensor_tensor(keys, keys, float(NSLOT),
                                               sx[:, 1:N + 1],
                                               op0=Alu.mult, op1=Alu.subtract)
                if STAGE < 3:
                    continue
                slots = small.tile([128, NSLOT], i16, tag="slots")
                nc.gpsimd.local_scatter(slots, iota1, keys, channels=128,
                                        num_elems=NSLOT, num_idxs=N)
                # slot (NSLOT-1-k) holds pos+1 of rank-k hit (k=1..16).
                # forward cols [NSLOT-17, NSLOT-1) = ranks 16..1 (reversed).
                sf = small.tile([128, 1], f32, tag="sf")
                nc.vector.tensor_copy(sf, sx[:, N:N + 1])        # min(c,...)+1
                cnt = small.tile([128, 1], f32, tag="cnt")
                nc.vector.tensor_scalar(cnt, sf, 1.0, 16.0,
                                        op0=Alu.subtract, op1=Alu.min)
                thr = small.tile([128, 1], f32, tag="thr")       # 16 - cnt
                nc.vector.tensor_scalar(thr, cnt, -1.0, 16.0,
                                        op0=Alu.mult, op1=Alu.add)
                slotsf = small.tile([128, KNN], f32, tag="slotsf")
                nc.vector.tensor_copy(slotsf,
                                      slots[:, NSLOT - 17:NSLOT - 1])
                idxf = small.tile([128, KNN], f32, tag="idxf")
                # col j valid iff j >= 16-cnt (rank 16-j <= cnt)
                nc.vector.scalar_tensor_tensor(idxf, iota16f, thr, slotsf,
                                               op0=Alu.is_ge, op1=Alu.mult)
                pad = small.tile([128, KNN], f32, tag="pad")
                nc.vector.scalar_tensor_tensor(
                    pad, iota16f, thr,
                    slotsf[:, KNN - 1:KNN].broadcast_to((128, KNN)),
                    op0=Alu.is_lt, op1=Alu.mult)
                nc.vector.tensor_tensor(idxf, idxf, pad, op=Alu.add)
                nc.vector.tensor_scalar_add(idxf, idxf, -1.0)
                nc.vector.tensor_copy(offs[:, t * KNN:(t + 1) * KNN], idxf)

            if STAGE < 5:
                for ch in range(SEQ):
                    nc.vector.tensor_copy(tacc2[:, ch:ch + 1], sx[:, N:N + 1])

            # ======== indirect gather + norms ========
            # partition p handles rows i = t*128+p; slot m = t*16+k.
            # One indirect DMA per slot column (one offset per partition).
            FM = NT * KNN
            gt = constp.tile([128, FM, SEQ * 3], f32)
            for m in range(FM if STAGE >= 5 else 0):
                nc.gpsimd.indirect_dma_start(
                    out=gt[:, m, :], out_offset=None, in_=table[:],
                    in_offset=bass.IndirectOffsetOnAxis(
                        ap=offs[:, m:m + 1], axis=0))
            for ch in range(SEQ if STAGE >= 5 else 0):
                Mc = 2 * KNN
                diff = gath.tile([128, 2, KNN, SEQ * 3], f32, tag="diff")
                nc.vector.tensor_tensor(
                    diff, gt.rearrange("p (t k) f -> p t k f", t=NT)
                            [:, 2 * ch:2 * ch + 2],
                    own[:, 2 * ch:2 * ch + 2, :]
                       .rearrange("p t (o f) -> p t o f", o=1)
                       .broadcast_to((128, 2, KNN, SEQ * 3)),
                    op=Alu.subtract)
                sq = gath.tile([128, 2, KNN, SEQ * 3], f32, tag="sq")
                nc.vector.tensor_tensor(sq, diff, diff, op=Alu.mult)
                q2 = gath.tile([128, 2 * KNN * SEQ], f32, tag="q2")
                nc.vector.reduce_sum(
                    q2.rearrange("p (a s) -> p a s", s=SEQ),
                    sq.rearrange("p t k (s c) -> p (t k) s c", c=3),
                    axis=mybir.AxisListType.X)
                dq = gath.tile([128, 2 * KNN * SEQ], f32, tag="dq")
                nc.scalar.activation(dq, q2, Act.Sqrt,
                                     accum_out=tacc2[:, ch:ch + 1])

            trow = constp.tile([128, 1], f32)
            nc.vector.reduce_sum(trow, tacc2, axis=mybir.AxisListType.X)
            tall = constp.tile([128, 1], f32)
            nc.gpsimd.partition_all_reduce(tall, trow, channels=128,
                                           reduce_op=bass_isa.ReduceOp.add)
            nc.sync.dma_start(partial[:], tall[:1, :])

    nc.finalize()
    return nc


def _get_program():
    if "nc" not in _CACHE:
        _CACHE["nc"] = _build_program()
    return _CACHE["nc"]


def _hi_lo(x32: np.ndarray):
    import ml_dtypes
    hi = x32.astype(ml_dtypes.bfloat16)
    lo = (x32 - hi.astype(np.float32)).astype(ml_dtypes.bfloat16)
    return hi, lo


def _aug_operands(pc: np.ndarray):
    """Build [16, N] rhs and per-core [16, SLAB] lhsT bf16 operand rows.

    Row pairing r: lhsT[r] * rhs[r] summed = d2 = |q|^2 + |s|^2 - 2 q.s
      r0-2: -2qh * sh   r3-5: -2qh * sl   r6-8: -2ql * sh   r9-11: -2ql * sl
      r12: qqh * 1      r13: qql * 1      r14: 1 * ssh      r15: 1 * ssl
    """
    import ml_dtypes
    bf = ml_dtypes.bfloat16
    xT = pc.T                                   # [3, N]
    sh, sl = _hi_lo(xT)
    ss = np.sum(pc.astype(np.float64) * pc, axis=1).astype(np.float32)
    ssh, ssl = _hi_lo(ss)
    rhs = np.zeros((KROWS, N), dtype=bf)
    rhs[0:3] = sh; rhs[3:6] = sl; rhs[6:9] = sh; rhs[9:12] = sl
    rhs[12:14] = np.ones((2, N), dtype=bf)
    rhs[14] = ssh; rhs[15] = ssl

    m2 = (-2.0 * xT).astype(np.float32)
    qh, ql = _hi_lo(m2)
    qqh, qql = _hi_lo(ss)
    lhsTs = []
    for c in range(NCORES):
        sl_ = slice(c * SLAB, (c + 1) * SLAB)
        l = np.zeros((KROWS, SLAB), dtype=bf)
        l[0:3] = qh[:, sl_]; l[3:6] = qh[:, sl_]
        l[6:9] = ql[:, sl_]; l[9:12] = ql[:, sl_]
        l[12] = qqh[sl_]; l[13] = qql[sl_]
        l[14:16] = np.ones((2, SLAB), dtype=bf)
        lhsTs.append(l)
    return rhs, lhsTs


def kernel(pc_source: np.ndarray, pred_flow: np.ndarray) -> np.ndarray:
    from concourse.bass_utils import run_bass_kernel_spmd

    nc = _get_program()
    pc = np.ascontiguousarray(np.asarray(pc_source)[0], dtype=np.float32)
    fl = np.ascontiguousarray(np.asarray(pred_flow), dtype=np.float32)
    rhs, lhsTs = _aug_operands(pc)
    in_maps = []
    for c in range(NCORES):
        sl = slice(c * SLAB, (c + 1) * SLAB)
        in_maps.append({
            "aug_rhs": rhs,
            "aug_lhsT": lhsTs[c],
            "flow_all": fl,
            "flow_slab": np.ascontiguousarray(fl[:, sl]),
        })
    res = run_bass_kernel_spmd(nc, in_maps, core_ids=list(range(NCORES)))
    total = np.sum([r["partial"][0, 0] for r in res.results], dtype=np.float64)
    return np.float32(total / (SEQ * N * KNN))


# revision 25
# speedup vs baseline: 1.0243x; 1.0243x over previous
"""Trainium2 Bass kernel for nn_BallQLossSeq (ball-query + grouped flow-norm loss).

Per core (1024 of 8192 query rows):
  1. PE: d2[i,j] via augmented matmul (16 contraction rows: hi/lo bf16 split of
     -2x, coords, |q|^2, |s|^2), 512-wide PSUM chunks.
  2. ACT: steep sigmoid (kappa=2^18) of (1-d2) -> ~0/1 hit indicator.
  3. DVE: tensor_tensor_scan (add, clamp via min 17) -> running hit count
     S in [0,17] as int16 keys.
  4. GPSIMD local_scatter: data = iota (t+1), keys = S -> slot[v] = position of
     hit v+1 (last-wins). Slots 0..15 = first-16 in-radius neighbor positions;
     rows with c<16 hits padded with slot[0] (first hit).
  5. dma_gather of 256B rows from a DRAM flow table, elementwise norm + sqrt,
     accumulate -> per-core scalar partial. Host sums partials / (S*N*K).
"""

import numpy as np

N = 8192
NCORES = 8
SLAB = N // NCORES          # 1024 query rows per core
NT = SLAB // 128            # 8 i-tiles per core
SEQ = 4
KNN = 16
NCHUNK = 16                 # j chunks of 512
CW = 512
KAPPA = 4194304.0
KROWS = 16                  # matmul contraction rows

_CACHE = {}


def _build_program():
    import os
    STAGE = int(os.environ.get("KSTAGE", "5"))
    import concourse.bass as bass
    import concourse.bacc as bacc
    import concourse.mybir as mybir
    import concourse.tile as tile
    import concourse.bass_isa as bass_isa

    f32 = mybir.dt.float32
    bf16 = mybir.dt.bfloat16
    i16 = mybir.dt.int16
    i32 = mybir.dt.int32
    Alu = mybir.AluOpType
    Act = mybir.ActivationFunctionType

    nc = bacc.Bacc()

    aug_rhs = nc.dram_tensor("aug_rhs", [KROWS, N], bf16, kind="ExternalInput")
    aug_lhsT = nc.dram_tensor("aug_lhsT", [KROWS, SLAB], bf16, kind="ExternalInput")
    flow_all = nc.dram_tensor("flow_all", [SEQ, N, 3], f32, kind="ExternalInput")
    flow_slab = nc.dram_tensor("flow_slab", [SEQ, SLAB, 3], f32, kind="ExternalInput")
    partial = nc.dram_tensor("partial", [1, 1], f32, kind="ExternalOutput")

    with tile.TileContext(nc) as tc:
        with (
            tc.tile_pool(name="const", bufs=1) as constp,
            tc.tile_pool(name="prep", bufs=1) as prep,
            tc.tile_pool(name="hpool", bufs=3) as hpool,
            tc.tile_pool(name="kpool", bufs=3) as kpool,
            tc.tile_pool(name="small", bufs=2) as small,
            tc.tile_pool(name="gath", bufs=2) as gath,
            tc.tile_pool(name="dram", bufs=1, space="DRAM") as drampool,
            tc.tile_pool(name="psum", bufs=6, space="PSUM") as psum,
            tc.tile_pool(name="tpsum", bufs=2, space="PSUM") as tpsum,
        ):
            # ---------------- constants ----------------
            iota1 = constp.tile([128, N], i16)           # values t+1
            nc.gpsimd.iota(iota1, pattern=[[1, N]], base=1, channel_multiplier=0)
            c17 = constp.tile([128, N], bf16)
            nc.gpsimd.memset(c17, 1792.0)
            iota16 = constp.tile([128, KNN], i32)
            nc.gpsimd.iota(iota16, pattern=[[1, KNN]], base=0, channel_multiplier=0)
            iota16f = constp.tile([128, KNN], f32)
            nc.vector.tensor_copy(iota16f, iota16)
            kbias = constp.tile([128, 1], f32)
            nc.gpsimd.memset(kbias, KAPPA)

            # ---------------- DRAM flow table [N, 12] (cols s*3+c) ------------
            table = drampool.tile([N, SEQ * 3], f32)
            for s in range(SEQ):
                nc.sync.dma_start(table[:, s * 3:(s + 1) * 3], flow_all[s])

            # ------------- aug matmul operands (host-prepped hi/lo bf16) ------
            rhs_t = constp.tile([KROWS, N], bf16)
            nc.sync.dma_start(rhs_t, aug_rhs[:])
            lhsT = constp.tile([KROWS, SLAB], bf16)
            nc.sync.dma_start(lhsT, aug_lhsT[:])

            # ------------- own flow vectors [128, NT, 12] (p = i%128) ----------
            own = constp.tile([128, NT, SEQ * 3], f32)
            for s in range(SEQ):
                nc.sync.dma_start(
                    own[:, :, 3 * s:3 * (s + 1)],
                    flow_slab[s].rearrange("(t p) c -> p t c", p=128))

            offs = constp.tile([128, NT * KNN], i32)
            tacc2 = constp.tile([128, SEQ], f32)

            # ================= main loop over i-tiles ==========================
            NSLOT = 1800
            for t in range(NT):
                h = hpool.tile([128, N], bf16, tag="h")
                for n in range(NCHUNK):
                    pd2 = psum.tile([128, CW], f32, tag="d2")
                    nc.tensor.matmul(pd2, lhsT[:, t * 128:(t + 1) * 128],
                                     rhs_t[:, n * CW:(n + 1) * CW],
                                     start=True, stop=True)
                    # h = sigmoid(kappa*(1 - d2))
                    nc.scalar.activation(h[:, n * CW:(n + 1) * CW], pd2,
                                         Act.Sigmoid, bias=kbias[:, :],
                                         scale=-KAPPA)
                # S[t] = min(1 + cumsum(h), 1792), with S[-1]=1 prepended
                # chunk-chained scan: DVE trails ACT chunk-by-chunk.
                # S = min(1 + cumsum(h), 1792); key = 1800*h - S
                # (hit -> unique slot 1800-S; miss -> negative, ignored)
                sx = kpool.tile([128, N + 8], i32, tag="sx", bufs=1)
                keys = kpool.tile([128, N], i16, tag="keys")
                if STAGE < 2:
                    continue
                for n2 in range(NCHUNK):
                    lo, hi2 = n2 * CW, (n2 + 1) * CW
                    init = 1.3 if n2 == 0 else sx[:, lo:lo + 1]
                    nc.vector.tensor_tensor_scan(
                        sx[:, lo + 1:hi2 + 1], h[:, lo:hi2], c17[:, lo:hi2],
                        initial=init, op0=Alu.add, op1=Alu.min)
                    nc.vector.scalar_tensor_tensor(
                        keys[:, lo:hi2], h[:, lo:hi2], float(NSLOT),
                        sx[:, lo + 1:hi2 + 1], op0=Alu.mult,
                        op1=Alu.subtract)
                if STAGE < 3:
                    continue
                slots = small.tile([128, NSLOT], i16, tag="slots")
                nc.gpsimd.local_scatter(slots, iota1, keys, channels=128,
                                        num_elems=NSLOT, num_idxs=N)
                # slot (NSLOT-1-k) holds pos+1 of rank-k hit (k=1..16).
                # forward cols [NSLOT-17, NSLOT-1) = ranks 16..1 (reversed).
                sf = small.tile([128, 1], f32, tag="sf")
                nc.vector.tensor_copy(sf, sx[:, N:N + 1])        # min(c,...)+1
                cnt = small.tile([128, 1], f32, tag="cnt")
                nc.vector.tensor_scalar(cnt, sf, 1.0, 16.0,
                                        op0=Alu.subtract, op1=Alu.min)
                thr = small.tile([128, 1], f32, tag="thr")       # 16 - cnt
                nc.vector.tensor_scalar(thr, cnt, -1.0, 16.0,
                                        op0=Alu.mult, op1=Alu.add)
                slotsf = small.tile([128, KNN], f32, tag="slotsf")
                nc.vector.tensor_copy(slotsf,
                                      slots[:, NSLOT - 17:NSLOT - 1])
                idxf = small.tile([128, KNN], f32, tag="idxf")
                # col j valid iff j >= 16-cnt (rank 16-j <= cnt)
                nc.vector.scalar_tensor_tensor(idxf, iota16f, thr, slotsf,
                                               op0=Alu.is_ge, op1=Alu.mult)
                pad = small.tile([128, KNN], f32, tag="pad")
                nc.vector.scalar_tensor_tensor(
                    pad, iota16f, thr,
                    slotsf[:, KNN - 1:KNN].broadcast_to((128, KNN)),
                    op0=Alu.is_lt, op1=Alu.mult)
                nc.vector.tensor_tensor(idxf, idxf, pad, op=Alu.add)
                nc.vector.tensor_scalar_add(idxf, idxf, -1.0)
                nc.vector.tensor_copy(offs[:, t * KNN:(t + 1) * KNN], idxf)

            if STAGE < 5:
                for ch in range(SEQ):
                    nc.vector.tensor_copy(tacc2[:, ch:ch + 1], sx[:, N:N + 1])

            # ======== indirect gather + norms ========
            # partition p handles rows i = t*128+p; slot m = t*16+k.
            # One indirect DMA per slot column (one offset per partition).
            FM = NT * KNN
            gt = constp.tile([128, FM, SEQ * 3], f32)
            for m in range(FM if STAGE >= 5 else 0):
                nc.gpsimd.indirect_dma_start(
                    out=gt[:, m, :], out_offset=None, in_=table[:],
                    in_offset=bass.IndirectOffsetOnAxis(
                        ap=offs[:, m:m + 1], axis=0))
            for ch in range(SEQ if STAGE >= 5 else 0):
                Mc = 2 * KNN
                diff = gath.tile([128, 2, KNN, SEQ * 3], f32, tag="diff")
                nc.vector.tensor_tensor(
                    diff, gt.rearrange("p (t k) f -> p t k f", t=NT)
                            [:, 2 * ch:2 * ch + 2],
                    own[:, 2 * ch:2 * ch + 2, :]
                       .rearrange("p t (o f) -> p t o f", o=1)
                       .broadcast_to((128, 2, KNN, SEQ * 3)),
                    op=Alu.subtract)
                sq = gath.tile([128, 2, KNN, SEQ * 3], f32, tag="sq")
                nc.vector.tensor_tensor(sq, diff, diff, op=Alu.mult)
                q2 = gath.tile([128, 2 * KNN * SEQ], f32, tag="q2")
                nc.vector.reduce_sum(
                    q2.rearrange("p (a s) -> p a s", s=SEQ),
                    sq.rearrange("p t k (s c) -> p (t k) s c", c=3),
                    axis=mybir.AxisListType.X)
                dq = gath.tile([128, 2 * KNN * SEQ], f32, tag="dq")
                nc.scalar.activation(dq, q2, Act.Sqrt,
                                     accum_out=tacc2[:, ch:ch + 1])

            trow = constp.tile([128, 1], f32)
            nc.vector.reduce_sum(trow, tacc2, axis=mybir.AxisListType.X)
            tall = constp.tile([128, 1], f32)
            nc.gpsimd.partition_all_reduce(tall, trow, channels=128,
                                           reduce_op=bass_isa.ReduceOp.add)
            nc.sync.dma_start(partial[:], tall[:1, :])

    nc.finalize()
    return nc


def _get_program():
    if "nc" not in _CACHE:
        _CACHE["nc"] = _build_program()
    return _CACHE["nc"]


def _hi_lo(x32: np.ndarray):
    import ml_dtypes
    hi = x32.astype(ml_dtypes.bfloat16)
    lo = (x32 - hi.astype(np.float32)).astype(ml_dtypes.bfloat16)
    return hi, lo


def _aug_operands(pc: np.ndarray):
    """Build [16, N] rhs and per-core [16, SLAB] lhsT bf16 operand rows.

    Row pairing r: lhsT[r] * rhs[r] summed = d2 = |q|^2 + |s|^2 - 2 q.s
      r0-2: -2qh * sh   r3-5: -2qh * sl   r6-8: -2ql * sh   r9-11: -2ql * sl
      r12: qqh * 1      r13: qql * 1      r14: 1 * ssh      r15: 1 * ssl
    """
    import ml_dtypes
    bf = ml_dtypes.bfloat16
    xT = pc.T                                   # [3, N]
    sh, sl = _hi_lo(xT)
    ss = np.sum(pc.astype(np.float64) * pc, axis=1).astype(np.float32)
    ssh, ssl = _hi_lo(ss)
    rhs = np.zeros((KROWS, N), dtype=bf)
    rhs[0:3] = sh; rhs[3:6] = sl; rhs[6:9] = sh; rhs[9:12] = sl
    rhs[12:14] = np.ones((2, N), dtype=bf)
    rhs[14] = ssh; rhs[15] = ssl

    m2 = (-2.0 * xT).astype(np.float32)
    qh, ql = _hi_lo(m2)
    qqh, qql = _hi_lo(ss)
    lhsTs = []
    for c in range(NCORES):
        sl_ = slice(c * SLAB, (c + 1) * SLAB)
        l = np.zeros((KROWS, SLAB), dtype=bf)
        l[0:3] = qh[:, sl_]; l[3:6] = qh[:, sl_]
        l[6:9] = ql[:, sl_]; l[9:12] = ql[:, sl_]
        l[12] = qqh[sl_]; l[13] = qql[sl_]
        l[14:16] = np.ones((2, SLAB), dtype=bf)
        lhsTs.append(l)
    return rhs, lhsTs


def kernel(pc_source: np.ndarray, pred_flow: np.ndarray) -> np.ndarray:
    from concourse.bass_utils import run_bass_kernel_spmd

    nc = _get_program()
    pc = np.ascontiguousarray(np.asarray(pc_source)[0], dtype=np.float32)
    fl = np.ascontiguousarray(np.asarray(pred_flow), dtype=np.float32)
    rhs, lhsTs = _aug_operands(pc)
    in_maps = []
    for c in range(NCORES):
        sl = slice(c * SLAB, (c + 1) * SLAB)
        in_maps.append({
            "aug_rhs": rhs,
            "aug_lhsT": lhsTs[c],
            "flow_all": fl,
            "flow_slab": np.ascontiguousarray(fl[:, sl]),
        })
    res = run_bass_kernel_spmd(nc, in_maps, core_ids=list(range(NCORES)))
    total = np.sum([r["partial"][0, 0] for r in res.results], dtype=np.float64)
    return np.float32(total / (SEQ * N * KNN))
